# revision 23
# baseline (speedup 1.0000x reference)
"""Trainium2 Bass kernel for nn_HGraphAttentionLayer (GAT-style layer, 8 NeuronCores).

Math (reference):
  feats[h,n,o]  = concat(input[:5000] @ proj_rna[h], input[5000:] @ proj_dis[h])
  s_src[h,n]    = feats[h,n,:] @ score_src[h];  s_tgt likewise
  attn[h,i,j]   = softmax_over_i( mask[i,j] + leaky_relu(s_src[h,i]+s_tgt[h,j], 0.2) )
  vals[i,o]     = mean_h( sum_j attn[h,i,j] * feats[h,j,o] )
  out           = elu( instancenorm(vals) + input @ residual_w.T )

Sharding: each of the 8 cores owns N/8 = 1024 query rows (i). The mask arrives
host-transposed to [j, i] bf16 so each e-tile [128j, 1024i] is a straight DMA.
A custom fused DVE op computes y = leaky(mask + a_i + b_j) in one 1-cyc/elem
pass; ACT computes e = exp(y) with the d-partial as free accumulation. d is
completed with one small AllReduce per j-stripe; the bmm contracts e against
feats/d with vals^T accumulating in PSUM across all stripes.
"""
import numpy as np

N, F, H, O = 8192, 256, 4, 128
N_CORES = 8
MY_N = N // N_CORES          # 1024 rows per core
N_RNA = 5000
SLOPE = 0.2
EPS = 1e-5
N_STRIPES = 8
SJ = N // N_STRIPES          # 1024 j per stripe
JT = SJ // 128               # 8 j-tiles per stripe
NCH = N // 128               # 64 n-chunks
FC = F // 128                # 2 f-chunks
SPLIT_CH = N_RNA // 128      # chunk 39 contains the rna/dis boundary
SPLIT_ROW = N_RNA - SPLIT_CH * 128  # row 8 within chunk 39

_cached = {}


def _leaky_2x_uop():
    """Hand-written 2X_1PORT micro-op: both packed bf16 elements of each
    32-bit port read are computed per cycle (element A in blocks 0-3,
    element B in blocks 4-7; A's result rides delay lane 0 to the packed
    write). Mirrors the stock TensorTensor 2x program's structure."""
    from concourse.dve_uop import (
        UopConfig, UopDpConfig, InpSel, OutSel, AluOp, AluInp, DelayInp,
        OutPath, Trigger,
    )

    PD = DelayInp.PREV_DELAY if hasattr(DelayInp, "PREV_DELAY") else DelayInp(5)
    PA = DelayInp.PREV_ALU_OUT

    def blk(op, s0, s1, delay_sel, delay_en):
        d = [PD] * 7
        for i, v in delay_sel.items():
            d[i] = v
        en = [0] * 7
        for i in delay_en:
            en[i] = 1
        return UopDpConfig(op=op, alu_src0=AluInp(s0), alu_src1=AluInp(s1),
                           delay=d, alu_out_enable=1, delay_enable=en)

    A = AluInp
    dp = [
        # A: s = src0 + src1
        blk(AluOp.ADD, A.PREV_DELAY_0, A.PREV_DELAY_1, {}, [2, 3, 4, 5]),
        # A: t = s + C0(b)
        blk(AluOp.ADD, A.PREV_ALU_OUT, A.PREV_DELAY_4, {}, [2, 3, 4, 5]),
        # A: u = t * C2(0.2); lane0 <- t
        blk(AluOp.MULTIPLY, A.PREV_ALU_OUT, A.PREV_DELAY_5,
            {0: PA}, [0, 2, 3, 4, 5]),
        # A: y = max(u, t)
        blk(AluOp.MAX, A.PREV_ALU_OUT, A.PREV_DELAY_0, {}, [2, 3, 4, 5]),
        # B: s = src0_hi + src1_hi; lane0 <- yA
        blk(AluOp.ADD, A.PREV_DELAY_2, A.PREV_DELAY_3, {0: PA}, [0, 4, 5]),
        # B: t = s + C0
        blk(AluOp.ADD, A.PREV_ALU_OUT, A.PREV_DELAY_4, {}, [0, 5]),
        # B: u = t * C2; lane1 <- t
        blk(AluOp.MULTIPLY, A.PREV_ALU_OUT, A.PREV_DELAY_5, {1: PA}, [0, 1]),
        # B: y = max(u, t)
        blk(AluOp.MAX, A.PREV_ALU_OUT, A.PREV_DELAY_1, {}, [0]),
    ]
    return UopConfig(
        inp=[InpSel.ZERO, InpSel.SRC_0, InpSel.SRC_1, InpSel.SRC_0_HI,
             InpSel.SRC_1_HI, InpSel.CONST_0, InpSel.CONST_2, InpSel.ZERO],
        inp_enable=[0, 1, 1, 1, 1, 1, 1, 0],
        out={OutPath.WR0_LO: OutSel.DELAY_0, OutPath.WR0_HI: OutSel.ALU_OUT,
             OutPath.WR1_LO: OutSel.ALU_OUT, OutPath.WR1_HI: OutSel.ALU_OUT},
        out_enable={OutPath.WR0_LO: 1, OutPath.WR0_HI: 1,
                    OutPath.WR1_LO: 0, OutPath.WR1_HI: 0},
        require_inp0=1,
        require_inp1=1,
        trigger=(Trigger.SRC_TENSOR_DONE, Trigger.NONE, Trigger.NONE),
        datapath_config=dp,
    )


def _register_masked_leaky():
    """Runtime-register the fused DVE op  y = max(t, 0.2*t), t = in0+in1+s0.
    in0 = mask^T tile, in1 = a_i broadcast, s0 = b_j per-partition column.
    A hand-written 2X_1PORT variant doubles bf16 throughput."""
    from concourse import dve_ops
    from concourse.dve_ops import DveOp, OPS
    from concourse.dve_spec import Spec, Src0, Src1, C0, C2, maxx, lower
    from concourse.dve_uop import DveOpSpec
    from concourse.dve_table_gen import dve_ver_for

    for op in OPS:
        if op.name == "MASKED_LEAKY_ATTN":
            return op

    t = Src0 + Src1 + C0
    spec = Spec(
        body=maxx(t, t * C2),
        reference=lambda in0, in1, s0, s1, imm2: (
            lambda tt: np.maximum(tt, tt * imm2)
        )(np.asarray(in0, np.float32) + np.asarray(in1, np.float32) + s0),
    )
    op = DveOp("MASKED_LEAKY_ATTN", spec, subdim=False, uops_sha={})
    OPS.append(op)
    row = dve_ops._CUSTOM_DVE_ROW_BASE + len(OPS) - 1
    assert row < 0x20
    dve_ops._SUB_OPCODE_FOR_NAME[op.name] = row
    dve_ops.CUSTOM_DVE_SPECS[op.name] = spec
    for trn in ("TRN2",):
        ver = dve_ver_for(trn)
        lowered = DveOpSpec(
            name=op.name,
            opcode=row,
            uops=lower(spec, ver=ver),
            uops_2x=[_leaky_2x_uop()],
            perf_max=1,
            rd1_en=True,
        )
        lowered.validate(ver)
        op.uops_sha[ver] = lowered.sha(ver)
        dve_ops._COMPILE_CACHE[(op.name, ver)] = lowered
    return op


def _build():
    import concourse.bass as bass
    import concourse.bacc as bacc
    import concourse.mybir as mybir
    import concourse.tile as tile

    f32 = mybir.dt.float32
    bf16 = mybir.dt.bfloat16
    Alu = mybir.AluOpType
    Act = mybir.ActivationFunctionType

    LEAKY_OP = _register_masked_leaky()

    nc = bacc.Bacc("TRN2", target_bir_lowering=False, debug=False,
                   enable_asserts=False, num_devices=N_CORES)

    # ---- I/O (host pre-transposed / bf16 where layout-only) -------------
    maskT = nc.dram_tensor("maskT", [N, MY_N], bf16, kind="ExternalInput").ap()
    inT = nc.dram_tensor("inT", [FC, 128, N], bf16, kind="ExternalInput").ap()
    in_rnaT = nc.dram_tensor("in_rnaT", [FC, 128, MY_N], bf16, kind="ExternalInput").ap()
    in_disT = nc.dram_tensor("in_disT", [FC, 128, MY_N], bf16, kind="ExternalInput").ap()
    projc_rna = nc.dram_tensor("projc_rna", [FC, 128, H * O], bf16, kind="ExternalInput").ap()
    projc_dis = nc.dram_tensor("projc_dis", [FC, 128, H * O], bf16, kind="ExternalInput").ap()
    score_src = nc.dram_tensor("score_src", [H, O, 1], f32, kind="ExternalInput").ap()
    score_tgt = nc.dram_tensor("score_tgt", [H, O, 1], f32, kind="ExternalInput").ap()
    wrT_in = nc.dram_tensor("wrT", [FC, 128, O], bf16, kind="ExternalInput").ap()
    identf_in = nc.dram_tensor("identf", [128, 128], f32, kind="ExternalInput").ap()
    sel39_in = nc.dram_tensor("sel39", [128, 1], f32, kind="ExternalInput").ap()
    invsel39_in = nc.dram_tensor("invsel39", [128, 1], f32, kind="ExternalInput").ap()
    out_dram = nc.dram_tensor("out", [O, MY_N], f32, kind="ExternalOutput").ap()

    RG = [list(range(N_CORES))]

    with tile.TileContext(nc) as tc:
        with (
            tc.tile_pool(name="const", bufs=1) as constp,
            tc.tile_pool(name="pro", bufs=3) as pro,
            tc.tile_pool(name="dpool", bufs=3) as dpool,
            tc.tile_pool(name="ps_feat", bufs=2, space="PSUM") as ps_feat,
            tc.tile_pool(name="ps_s", bufs=2, space="PSUM") as ps_s,
            tc.tile_pool(name="ps_vals", bufs=1, space="PSUM") as ps_vals,
            tc.tile_pool(name="dram", bufs=1, space="DRAM") as dram,
        ):
            # ---- DRAM scratch ------------------------------------------
            feats_dram = dram.tile([NCH, 128, H * O], bf16, tag="featsd", name="featsd")
            # d is reduced at half-stripe granularity (4 j-tiles per unit,
            # 16 units) so the bmm trails the e-compute by only one stripe
            # half; cols are laid out jt*4+h so halves are contiguous.
            N_UNITS = 2 * N_STRIPES
            d_in = [dram.tile([128, 16], f32, tag=f"din{u}", name=f"din{u}")
                    for u in range(N_UNITS)]
            d_out = [dram.tile([128, 16], f32, tag=f"dout{u}", name=f"dout{u}")
                     for u in range(N_UNITS)]
            st_in = dram.tile([1, 32], f32, tag="stin", name="stin")
            st_out = dram.tile([1, 32], f32, tag="stout", name="stout")
            dum_in = dram.tile([128, 32], f32, tag="dumin", name="dumin")
            dum_out = dram.tile([128, 32], f32, tag="dumout", name="dumout")
            arow_dram = dram.tile([H, MY_N], f32, tag="arowd", name="arowd")

            # warm up the collective stack before ANY other DMA is queued
            # (one-time comm init + first-transfer latency ~100us otherwise
            # lands on stripe 0's d AllReduce; its data DMA must not sit
            # behind bulk mask/feats descriptors)
            zr = constp.tile([128, 32], f32, tag="zr", name="zr")
            nc.vector.memset(zr[:], 0.0)
            nc.sync.dma_start(dum_in[:], zr[:])
            nc.gpsimd.collective_compute(
                "AllReduce", Alu.add, replica_groups=RG,
                ins=[dum_in.opt()], outs=[dum_out.opt()])

            # ---- constants ---------------------------------------------
            identf = constp.tile([128, 128], f32, tag="identf", name="identf")
            nc.sync.dma_start(identf[:], identf_in)
            ones_col = constp.tile([128, 1], f32, tag="ones_col", name="ones_col")
            nc.vector.memset(ones_col[:], 1.0)
            ones_row = constp.tile([1, 512], f32, tag="ones_row", name="ones_row")
            nc.vector.memset(ones_row[:], 1.0)
            sel39 = constp.tile([128, 1], f32, tag="sel39", name="sel39")
            nc.sync.dma_start(sel39[:], sel39_in)
            invsel39 = constp.tile([128, 1], f32, tag="invsel39", name="invsel39")
            nc.sync.dma_start(invsel39[:], invsel39_in)

            # ---- resident SBUF inputs ----------------------------------
            rnaT = [constp.tile([128, MY_N], bf16, tag=f"rnaT{fc}", name=f"rnaT{fc}")
                    for fc in range(FC)]
            disT = [constp.tile([128, MY_N], bf16, tag=f"disT{fc}", name=f"disT{fc}")
                    for fc in range(FC)]
            for fc in range(FC):
                nc.sync.dma_start(rnaT[fc][:], in_rnaT[fc])
                nc.sync.dma_start(disT[fc][:], in_disT[fc])
            projb = {}
            for tname, pap in (("rna", projc_rna), ("dis", projc_dis)):
                for fc in range(FC):
                    pb = constp.tile([128, H * O], bf16, tag=f"pb_{tname}{fc}",
                                     name=f"pb_{tname}{fc}")
                    nc.sync.dma_start(pb[:], pap[fc])
                    projb[(tname, fc)] = pb
            wrT = []
            for fc in range(FC):
                wt = constp.tile([128, O], bf16, tag=f"wrT{fc}", name=f"wrT{fc}")
                nc.sync.dma_start(wt[:], wrT_in[fc])
                wrT.append(wt)

            # ---- score vectors -> q[type][fc] = [128f, 8] bf16 ----------
            # cols 0..3 = src head h, 4..7 = tgt head h
            q_rhs = {(t, fc): constp.tile([128, 8], bf16, tag=f"q{t}{fc}", name=f"q{t}{fc}")
                     for t in ("rna", "dis") for fc in range(FC)}
            for si, sap in ((0, score_src), (1, score_tgt)):
                for h in range(H):
                    scol = pro.tile([128, 1], f32, tag="scol", name="scol", bufs=2)
                    nc.sync.dma_start(scol[:], sap[h])
                    tpq = ps_s.tile([128, 128], f32, tag="tpq", name="tpq")
                    nc.tensor.transpose(tpq[0:1, :], scol[:], identf[:])
                    wrow = pro.tile([1, 128], f32, tag="wrow", name="wrow", bufs=2)
                    nc.vector.tensor_copy(wrow[:], tpq[0:1, :])
                    wb = pro.tile([128, 128], f32, tag="wb", name="wb", bufs=2)
                    nc.gpsimd.partition_broadcast(wb[:], wrow[:])
                    for tname in ("rna", "dis"):
                        for fc in range(FC):
                            qcol = pro.tile([128, 1], f32, tag="qcol", name="qcol", bufs=2)
                            qscr = pro.tile([128, O], f32, tag="qscr", name="qscr", bufs=2)
                            nc.vector.scalar_tensor_tensor(
                                qscr[:], projb[(tname, fc)][:, h * 128:(h + 1) * 128],
                                1.0, wb[:],
                                op0=Alu.mult, op1=Alu.mult, accum_out=qcol[:])
                            nc.vector.tensor_copy(
                                q_rhs[(tname, fc)][:, si * 4 + h:si * 4 + h + 1], qcol[:])

            # ---- s for my rows -> A_bcast[h] = [128, MY_N] bf16 ---------
            for ic in range(MY_N // 128):
                ps_sr = ps_s.tile([128, 8], f32, tag="small", name="pssr")
                k = 0
                for tname, Tt in (("rna", rnaT), ("dis", disT)):
                    for fc in range(FC):
                        nc.tensor.matmul(ps_sr[:], Tt[fc][:, ic * 128:(ic + 1) * 128],
                                         q_rhs[(tname, fc)][:],
                                         start=(k == 0), stop=(k == 3))
                        k += 1
                srow = pro.tile([128, 8], f32, tag="srow", name="srow", bufs=2)
                nc.vector.tensor_copy(srow[:], ps_sr[:])
                tps = ps_s.tile([128, 128], f32, tag="tpq", name="tps")
                nc.tensor.transpose(tps[0:8, :], srow[:], identf[:])
                srT = pro.tile([8, 128], f32, tag="srT", name="srT", bufs=2)
                nc.vector.tensor_copy(srT[:], tps[0:8, :])
                for h in range(H):
                    nc.sync.dma_start(arow_dram[h, ic * 128:(ic + 1) * 128], srT[h:h + 1, :])
            A_bcast = []
            for h in range(H):
                af = pro.tile([128, MY_N], f32, tag="af", name="af", bufs=1)
                nc.sync.dma_start(af[:], arow_dram[h:h + 1, :].partition_broadcast(128))
                ab = constp.tile([128, MY_N], bf16, tag=f"ab{h}", name=f"ab{h}")
                nc.vector.tensor_copy(ab[:], af[:])
                A_bcast.append(ab)

            # ---- full feats + s_all (shared inT chunk loop, PE work) ----
            # feats[ch] spilled to DRAM bf16; s_all[ch] kept in SBUF. The
            # chunk loop is spread through the stripe loop (stripe s emits
            # the chunks stripe s+2 will need) so the DVE/ACT queues reach
            # stripe 0's elementwise work early.
            s_all = [constp.tile([128, 8], f32, tag=f"sall{ch}", name=f"sall{ch}")
                     for ch in range(NCH)]

            def chunk_parts(ch):
                if ch < SPLIT_CH:
                    return ["rna"]
                if ch > SPLIT_CH:
                    return ["dis"]
                return ["rna", "dis"]

            def emit_chunk(ch):
                inTs = []
                for fc in range(FC):
                    itl = pro.tile([128, 128], bf16, tag="inT_ld", name="inT_ld", bufs=6)
                    nc.sync.dma_start(itl[:], inT[fc, :, ch * 128:(ch + 1) * 128])
                    inTs.append(itl)
                tmpf, tmps = {}, {}
                for tname in chunk_parts(ch):
                    ps_f = ps_feat.tile([128, H * O], f32, tag="psf", name="psf")
                    ps_sc = ps_s.tile([128, 8], f32, tag="small", name="pssc")
                    for fc in range(FC):
                        nc.tensor.matmul(ps_f[:], inTs[fc][:], projb[(tname, fc)][:],
                                         start=(fc == 0), stop=(fc == FC - 1))
                    for fc in range(FC):
                        nc.tensor.matmul(ps_sc[:], inTs[fc][:], q_rhs[(tname, fc)][:],
                                         start=(fc == 0), stop=(fc == FC - 1))
                    tmpf[tname] = ps_f
                    tmps[tname] = ps_sc
                if ch != SPLIT_CH:
                    tname = chunk_parts(ch)[0]
                    fsb = pro.tile([128, H * O], bf16, tag="fsb", name="fsb", bufs=4)
                    nc.scalar.activation(fsb[:], tmpf[tname][:], Act.Copy)
                    nc.sync.dma_start(feats_dram[ch], fsb[:])
                    nc.vector.tensor_copy(s_all[ch][:], tmps[tname][:])
                else:
                    # row-wise blend: rows < SPLIT_ROW take rna, rest take dis
                    t1f = pro.tile([128, H * O], f32, tag="blendf", name="blendf", bufs=1)
                    nc.vector.tensor_scalar_mul(t1f[:], tmpf["dis"][:], invsel39[:])
                    fb = pro.tile([128, H * O], bf16, tag="fb39", name="fb39", bufs=1)
                    nc.vector.scalar_tensor_tensor(
                        fb[:], tmpf["rna"][:], sel39[:], t1f[:],
                        op0=Alu.mult, op1=Alu.add)
                    nc.sync.dma_start(feats_dram[ch], fb[:])
                    t1s = pro.tile([128, 8], f32, tag="blends", name="blends", bufs=1)
                    nc.vector.tensor_scalar_mul(t1s[:], tmps["dis"][:], invsel39[:])
                    nc.vector.scalar_tensor_tensor(
                        s_all[ch][:], tmps["rna"][:], sel39[:], t1s[:],
                        op0=Alu.mult, op1=Alu.add)

            # ---- main loop over j-stripes (software-pipelined) ---------
            mpool = tc.alloc_tile_pool(name="mpool", bufs=4)
            ypool = tc.alloc_tile_pool(name="ypool", bufs=4)
            epool = tc.alloc_tile_pool(name="epool", bufs=3)
            gpool = tc.alloc_tile_pool(name="gpool", bufs=3)
            vals_ps = ps_vals.tile([128, MY_N], f32, tag="big", name="vals")
            d_alls = {}
            e_tiles = {}

            HJT = JT // 2            # 4 j-tiles per unit
            N_UNITS = 2 * N_STRIPES  # 16 units

            def emit_e_unit(u):
                # one unit = 4 j-tiles x 4 heads, own d AllReduce
                d_all = dpool.tile([128, 16], f32, tag="dall", name="dall")
                for k in range(HJT):
                    ch = u * HJT + k
                    mT = mpool.tile([128, MY_N], bf16, tag="mT", name="mT")
                    nc.sync.dma_start(mT[:], maskT[ch * 128:(ch + 1) * 128, :])
                    for h in range(H):
                        y = ypool.tile([128, MY_N], bf16, tag="y", name="y")
                        nc.vector._custom_dve(
                            LEAKY_OP, out=y[:], in0=mT[:], in1=A_bcast[h][:],
                            s0=s_all[ch][:, 4 + h:5 + h], imm2=SLOPE)
                        e = epool.tile([128, MY_N], bf16, tag="e", name="e", bufs=70)
                        nc.scalar.activation(e[:], y[:], Act.Exp,
                                             accum_out=d_all[:, k * 4 + h:k * 4 + h + 1])
                        e_tiles[(u, h, k)] = e
                # complete d across cores (partial sums over i-rows)
                nc.sync.dma_start(d_in[u][:], d_all[:])
                nc.gpsimd.collective_compute(
                    "AllReduce", Alu.add, replica_groups=RG,
                    ins=[d_in[u].opt()], outs=[d_out[u].opt()])

            def emit_bmm_unit(u):
                # d(u) returned while the next units' e-compute runs, so the
                # DVE reciprocal below does not stall the queue.
                d_sum = dpool.tile([128, 16], f32, tag="dsum", name="dsum")
                nc.sync.dma_start(d_sum[:], d_out[u][:])
                dinv = dpool.tile([128, 16], f32, tag="dinv", name="dinv")
                nc.vector.reciprocal(dinv[:], d_sum[:])
                for k in range(HJT):
                    ch = u * HJT + k
                    fst4 = gpool.tile([128, H * O], bf16, tag="fst4", name="fst4")
                    nc.sync.dma_start(fst4[:], feats_dram[ch])
                    g4 = gpool.tile([128, H * O], bf16, tag="g4", name="g4")
                    for h in range(H):
                        nc.vector.tensor_scalar_mul(
                            g4[:, h * 128:(h + 1) * 128], fst4[:, h * 128:(h + 1) * 128],
                            dinv[:, k * 4 + h:k * 4 + h + 1])
                    for h in range(H):
                        e = e_tiles.pop((u, h, k))
                        first = u == 0 and h == 0 and k == 0
                        last = u == N_UNITS - 1 and h == H - 1 and k == HJT - 1
                        nc.tensor.matmul(vals_ps[:, 0:512], g4[:, h * 128:(h + 1) * 128],
                                         e[:, 0:512], start=first, stop=last)
                        nc.tensor.matmul(vals_ps[:, 512:1024], g4[:, h * 128:(h + 1) * 128],
                                         e[:, 512:1024], start=first, stop=last)

            for ch in range(JT):
                emit_chunk(ch)
            for u in range(N_UNITS):
                emit_e_unit(u)
                if u % 2 == 0 and JT + (u // 2) * JT < NCH:
                    s = u // 2
                    for jt in range(JT):
                        emit_chunk(JT + s * JT + jt)
                if u >= 1:
                    emit_bmm_unit(u - 1)
            emit_bmm_unit(N_UNITS - 1)

            # ---- tail: instance norm + residual + elu ------------------
            gpool.release()
            epool.release()
            ypool.release()
            mpool.release()
            tailp = tc.alloc_tile_pool(name="tail", bufs=1)
            vs = tailp.tile([128, MY_N], f32, tag="vs", name="vs")
            srow1 = tailp.tile([128, 1], f32, tag="srow1", name="srow1")
            nc.scalar.activation(vs[:], vals_ps[:], Act.Copy, scale=0.25,
                                 accum_out=srow1[:])
            vsq = tailp.tile([128, MY_N], f32, tag="vsq", name="vsq")
            srow2 = tailp.tile([128, 1], f32, tag="srow2", name="srow2")
            nc.scalar.activation(vsq[:], vs[:], Act.Square, accum_out=srow2[:])

            ps1 = ps_s.tile([1, 1], f32, tag="small", name="ps1")
            nc.tensor.matmul(ps1[:], srow1[:], ones_col[:])
            ps2 = ps_s.tile([1, 1], f32, tag="small", name="ps2")
            nc.tensor.matmul(ps2[:], srow2[:], ones_col[:])
            stv = tailp.tile([1, 32], f32, tag="stv", name="stv")
            nc.vector.memset(stv[:], 0.0)
            nc.vector.tensor_copy(stv[0:1, 0:1], ps1[:])
            nc.vector.tensor_copy(stv[0:1, 16:17], ps2[:])
            nc.sync.dma_start(st_in[:], stv[:])
            nc.gpsimd.collective_compute(
                "AllReduce", Alu.add, replica_groups=RG,
                ins=[st_in.opt()], outs=[st_out.opt()])
            str_ = tailp.tile([1, 32], f32, tag="str", name="str")
            nc.sync.dma_start(str_[:], st_out[:])

            c = 1.0 / float(N * O)
            mu = tailp.tile([1, 1], f32, tag="mu", name="mu")
            nc.vector.tensor_scalar_mul(mu[:], str_[0:1, 0:1], c)
            m2 = tailp.tile([1, 1], f32, tag="m2", name="m2")
            nc.vector.tensor_scalar_mul(m2[:], str_[0:1, 16:17], c)
            mu2 = tailp.tile([1, 1], f32, tag="mu2", name="mu2")
            nc.vector.tensor_mul(mu2[:], mu[:], mu[:])
            var = tailp.tile([1, 1], f32, tag="var", name="var")
            nc.vector.tensor_sub(var[:], m2[:], mu2[:])
            vpe = tailp.tile([1, 1], f32, tag="vpe", name="vpe")
            nc.vector.tensor_scalar_add(vpe[:], var[:], EPS)
            sd = tailp.tile([1, 1], f32, tag="sd", name="sd")
            nc.scalar.activation(sd[:], vpe[:], Act.Sqrt)
            rstd = tailp.tile([1, 1], f32, tag="rstd", name="rstd")
            nc.vector.reciprocal(rstd[:], sd[:])
            negmurs = tailp.tile([1, 1], f32, tag="negmurs", name="negmurs")
            nc.vector.tensor_mul(negmurs[:], mu[:], rstd[:])
            nc.vector.tensor_scalar_mul(negmurs[:], negmurs[:], -1.0)

            a_col = tailp.tile([128, 1], f32, tag="acol", name="acol")
            nc.gpsimd.partition_broadcast(a_col[:], rstd[:])
            b_row = tailp.tile([1, 128], f32, tag="brow", name="brow")
            nc.scalar.activation(b_row[:], ones_row[0:1, 0:128], Act.Copy,
                                 scale=negmurs[:])

            rowsT = [tailp.tile([128, MY_N], bf16, tag=f"rowsT{fc}", name=f"rowsT{fc}")
                     for fc in range(FC)]
            for fc in range(FC):
                nc.vector.tensor_add(rowsT[fc][:], rnaT[fc][:], disT[fc][:])

            r_ps = ps_vals.tile([128, MY_N], f32, tag="big", name="resid")
            for half in range(2):
                sl = slice(half * 512, (half + 1) * 512)
                for fc in range(FC):
                    nc.tensor.matmul(r_ps[:, sl], wrT[fc][:], rowsT[fc][:, sl],
                                     start=(fc == 0), stop=False)
                nc.tensor.matmul(r_ps[:, sl], b_row[:], ones_row[:],
                                 start=False, stop=True)

            pre = tailp.tile([128, MY_N], f32, tag="pre", name="pre")
            nc.vector.scalar_tensor_tensor(pre[:], vs[:], a_col[:], r_ps[:],
                                           op0=Alu.mult, op1=Alu.add)
            negp = tailp.tile([128, MY_N], f32, tag="negp", name="negp")
            nc.vector.tensor_scalar_min(negp[:], pre[:], 0.0)
            w = tailp.tile([128, MY_N], f32, tag="w", name="w")
            nc.scalar.activation(w[:], negp[:], Act.Exp)
            r1 = tailp.tile([128, MY_N], f32, tag="r1", name="r1")
            nc.vector.tensor_scalar_max(r1[:], pre[:], 0.0)
            outt = tailp.tile([128, MY_N], f32, tag="outt", name="outt")
            nc.vector.scalar_tensor_tensor(outt[:], w[:], -1.0, r1[:],
                                           op0=Alu.add, op1=Alu.add)
            nc.sync.dma_start(out_dram, outt[:])
            tailp.release()

    nc.compile()
    return nc


def _get_nc():
    if "nc" not in _cached:
        _cached["nc"] = _build()
    return _cached["nc"]


def kernel(input_mat, connectivity_mask, proj_rna, proj_dis, score_src,
           score_tgt, residual_w):
    import ml_dtypes
    from concourse.bass_utils import run_bass_kernel_spmd

    bf16 = ml_dtypes.bfloat16
    nc = _get_nc()
    input_mat = np.asarray(input_mat, np.float32)
    connectivity_mask = np.asarray(connectivity_mask, np.float32)
    proj_rna = np.asarray(proj_rna, np.float32)
    proj_dis = np.asarray(proj_dis, np.float32)
    residual_w = np.asarray(residual_w, np.float32)

    ident = np.eye(128, dtype=np.float32)
    sel39 = (np.arange(128) < SPLIT_ROW).astype(np.float32)[:, None]
    rna_mask = (np.arange(N) < N_RNA).astype(np.float32)[:, None]
    in_rna_full = input_mat * rna_mask
    in_dis_full = input_mat * (1.0 - rna_mask)

    # layout-only host prep: transposes + bf16 casts
    inT = np.ascontiguousarray(input_mat.T).reshape(FC, 128, N).astype(bf16)
    projc_rna = np.ascontiguousarray(
        np.transpose(proj_rna, (1, 0, 2)).reshape(F, H * O)
    ).reshape(FC, 128, H * O).astype(bf16)
    projc_dis = np.ascontiguousarray(
        np.transpose(proj_dis, (1, 0, 2)).reshape(F, H * O)
    ).reshape(FC, 128, H * O).astype(bf16)
    wrT = np.ascontiguousarray(residual_w.T).reshape(FC, 128, O).astype(bf16)

    in_maps = []
    for k in range(N_CORES):
        r0, r1 = k * MY_N, (k + 1) * MY_N
        in_maps.append({
            "maskT": np.ascontiguousarray(connectivity_mask[r0:r1].T).astype(bf16),
            "inT": inT,
            "in_rnaT": np.ascontiguousarray(in_rna_full[r0:r1].T).reshape(
                FC, 128, MY_N).astype(bf16),
            "in_disT": np.ascontiguousarray(in_dis_full[r0:r1].T).reshape(
                FC, 128, MY_N).astype(bf16),
            "projc_rna": projc_rna,
            "projc_dis": projc_dis,
            "score_src": np.asarray(score_src, np.float32),
            "score_tgt": np.asarray(score_tgt, np.float32),
            "wrT": wrT,
            "identf": ident,
            "sel39": sel39,
            "invsel39": 1.0 - sel39,
        })

    res = run_bass_kernel_spmd(nc, in_maps, core_ids=list(range(N_CORES)))
    _cached["last_result"] = res
    out = np.empty((N, O), np.float32)
    for k in range(N_CORES):
        out[k * MY_N:(k + 1) * MY_N, :] = res.results[k]["out"].T
    return out


# revision 29
# speedup vs baseline: 1.0675x; 1.0675x over previous
"""Trainium2 Bass kernel for nn_HGraphAttentionLayer (GAT-style layer, 8 NeuronCores).

Math (reference):
  feats[h,n,o]  = concat(input[:5000] @ proj_rna[h], input[5000:] @ proj_dis[h])
  s_src[h,n]    = feats[h,n,:] @ score_src[h];  s_tgt likewise
  attn[h,i,j]   = softmax_over_i( mask[i,j] + leaky_relu(s_src[h,i]+s_tgt[h,j], 0.2) )
  vals[i,o]     = mean_h( sum_j attn[h,i,j] * feats[h,j,o] )
  out           = elu( instancenorm(vals) + input @ residual_w.T )

Sharding: each of the 8 cores owns N/8 = 1024 query rows (i). The mask arrives
host-transposed to [j, i] bf16 so each e-tile [128j, 1024i] is a straight DMA.
A custom fused DVE op computes y = leaky(mask + a_i + b_j) in one 1-cyc/elem
pass; ACT computes e = exp(y) with the d-partial as free accumulation. d is
completed with one small AllReduce per j-stripe; the bmm contracts e against
feats/d with vals^T accumulating in PSUM across all stripes.
"""
import numpy as np

N, F, H, O = 8192, 256, 4, 128
N_CORES = 8
MY_N = N // N_CORES          # 1024 rows per core
N_RNA = 5000
SLOPE = 0.2
EPS = 1e-5
N_STRIPES = 8
SJ = N // N_STRIPES          # 1024 j per stripe
JT = SJ // 128               # 8 j-tiles per stripe
NCH = N // 128               # 64 n-chunks
FC = F // 128                # 2 f-chunks
SPLIT_CH = N_RNA // 128      # chunk 39 contains the rna/dis boundary
SPLIT_ROW = N_RNA - SPLIT_CH * 128  # row 8 within chunk 39

_cached = {}


def _leaky_2x_uop():
    """Hand-written 2X_1PORT micro-op: both packed bf16 elements of each
    32-bit port read are computed per cycle (element A in blocks 0-3,
    element B in blocks 4-7; A's result rides delay lane 0 to the packed
    write). Mirrors the stock TensorTensor 2x program's structure."""
    from concourse.dve_uop import (
        UopConfig, UopDpConfig, InpSel, OutSel, AluOp, AluInp, DelayInp,
        OutPath, Trigger,
    )

    PD = DelayInp.PREV_DELAY if hasattr(DelayInp, "PREV_DELAY") else DelayInp(5)
    PA = DelayInp.PREV_ALU_OUT

    def blk(op, s0, s1, delay_sel, delay_en):
        d = [PD] * 7
        for i, v in delay_sel.items():
            d[i] = v
        en = [0] * 7
        for i in delay_en:
            en[i] = 1
        return UopDpConfig(op=op, alu_src0=AluInp(s0), alu_src1=AluInp(s1),
                           delay=d, alu_out_enable=1, delay_enable=en)

    A = AluInp
    dp = [
        # A: s = src0 + src1
        blk(AluOp.ADD, A.PREV_DELAY_0, A.PREV_DELAY_1, {}, [2, 3, 4, 5]),
        # A: t = s + C0(b)
        blk(AluOp.ADD, A.PREV_ALU_OUT, A.PREV_DELAY_4, {}, [2, 3, 4, 5]),
        # A: u = t * C2(0.2); lane0 <- t
        blk(AluOp.MULTIPLY, A.PREV_ALU_OUT, A.PREV_DELAY_5,
            {0: PA}, [0, 2, 3, 4, 5]),
        # A: y = max(u, t)
        blk(AluOp.MAX, A.PREV_ALU_OUT, A.PREV_DELAY_0, {}, [2, 3, 4, 5]),
        # B: s = src0_hi + src1_hi; lane0 <- yA
        blk(AluOp.ADD, A.PREV_DELAY_2, A.PREV_DELAY_3, {0: PA}, [0, 4, 5]),
        # B: t = s + C0
        blk(AluOp.ADD, A.PREV_ALU_OUT, A.PREV_DELAY_4, {}, [0, 5]),
        # B: u = t * C2; lane1 <- t
        blk(AluOp.MULTIPLY, A.PREV_ALU_OUT, A.PREV_DELAY_5, {1: PA}, [0, 1]),
        # B: y = max(u, t)
        blk(AluOp.MAX, A.PREV_ALU_OUT, A.PREV_DELAY_1, {}, [0]),
    ]
    return UopConfig(
        inp=[InpSel.ZERO, InpSel.SRC_0, InpSel.SRC_1, InpSel.SRC_0_HI,
             InpSel.SRC_1_HI, InpSel.CONST_0, InpSel.CONST_2, InpSel.ZERO],
        inp_enable=[0, 1, 1, 1, 1, 1, 1, 0],
        out={OutPath.WR0_LO: OutSel.DELAY_0, OutPath.WR0_HI: OutSel.ALU_OUT,
             OutPath.WR1_LO: OutSel.ALU_OUT, OutPath.WR1_HI: OutSel.ALU_OUT},
        out_enable={OutPath.WR0_LO: 1, OutPath.WR0_HI: 1,
                    OutPath.WR1_LO: 0, OutPath.WR1_HI: 0},
        require_inp0=1,
        require_inp1=1,
        trigger=(Trigger.SRC_TENSOR_DONE, Trigger.NONE, Trigger.NONE),
        datapath_config=dp,
    )


def _register_masked_leaky():
    """Runtime-register the fused DVE op  y = max(t, 0.2*t), t = in0+in1+s0.
    in0 = mask^T tile, in1 = a_i broadcast, s0 = b_j per-partition column.
    A hand-written 2X_1PORT variant doubles bf16 throughput."""
    from concourse import dve_ops
    from concourse.dve_ops import DveOp, OPS
    from concourse.dve_spec import Spec, Src0, Src1, C0, C2, maxx, lower
    from concourse.dve_uop import DveOpSpec
    from concourse.dve_table_gen import dve_ver_for

    for op in OPS:
        if op.name == "MASKED_LEAKY_ATTN":
            return op

    t = Src0 + Src1 + C0
    spec = Spec(
        body=maxx(t, t * C2),
        reference=lambda in0, in1, s0, s1, imm2: (
            lambda tt: np.maximum(tt, tt * imm2)
        )(np.asarray(in0, np.float32) + np.asarray(in1, np.float32) + s0),
    )
    op = DveOp("MASKED_LEAKY_ATTN", spec, subdim=False, uops_sha={})
    OPS.append(op)
    row = dve_ops._CUSTOM_DVE_ROW_BASE + len(OPS) - 1
    assert row < 0x20
    dve_ops._SUB_OPCODE_FOR_NAME[op.name] = row
    dve_ops.CUSTOM_DVE_SPECS[op.name] = spec
    for trn in ("TRN2",):
        ver = dve_ver_for(trn)
        lowered = DveOpSpec(
            name=op.name,
            opcode=row,
            uops=lower(spec, ver=ver),
            uops_2x=[_leaky_2x_uop()],
            perf_max=1,
            rd1_en=True,
        )
        lowered.validate(ver)
        op.uops_sha[ver] = lowered.sha(ver)
        dve_ops._COMPILE_CACHE[(op.name, ver)] = lowered
    return op


def _build():
    import concourse.bass as bass
    import concourse.bacc as bacc
    import concourse.mybir as mybir
    import concourse.tile as tile

    f32 = mybir.dt.float32
    bf16 = mybir.dt.bfloat16
    Alu = mybir.AluOpType
    Act = mybir.ActivationFunctionType

    LEAKY_OP = _register_masked_leaky()

    nc = bacc.Bacc("TRN2", target_bir_lowering=False, debug=False,
                   enable_asserts=False, num_devices=N_CORES)

    # ---- I/O (host pre-transposed / bf16 where layout-only) -------------
    maskT = nc.dram_tensor("maskT", [N, MY_N], bf16, kind="ExternalInput").ap()
    inT = nc.dram_tensor("inT", [FC, 128, N], bf16, kind="ExternalInput").ap()
    in_rnaT = nc.dram_tensor("in_rnaT", [FC, 128, MY_N], bf16, kind="ExternalInput").ap()
    in_disT = nc.dram_tensor("in_disT", [FC, 128, MY_N], bf16, kind="ExternalInput").ap()
    projc_rna = nc.dram_tensor("projc_rna", [FC, 128, H * O], bf16, kind="ExternalInput").ap()
    projc_dis = nc.dram_tensor("projc_dis", [FC, 128, H * O], bf16, kind="ExternalInput").ap()
    score_src = nc.dram_tensor("score_src", [H, O, 1], f32, kind="ExternalInput").ap()
    score_tgt = nc.dram_tensor("score_tgt", [H, O, 1], f32, kind="ExternalInput").ap()
    wrT_in = nc.dram_tensor("wrT", [FC, 128, O], bf16, kind="ExternalInput").ap()
    identf_in = nc.dram_tensor("identf", [128, 128], f32, kind="ExternalInput").ap()
    sel39_in = nc.dram_tensor("sel39", [128, 1], f32, kind="ExternalInput").ap()
    invsel39_in = nc.dram_tensor("invsel39", [128, 1], f32, kind="ExternalInput").ap()
    out_dram = nc.dram_tensor("out", [O, MY_N], f32, kind="ExternalOutput").ap()

    RG = [list(range(N_CORES))]

    with tile.TileContext(nc) as tc:
        with (
            tc.tile_pool(name="const", bufs=1) as constp,
            tc.tile_pool(name="pro", bufs=3) as pro,
            tc.tile_pool(name="dpool", bufs=3) as dpool,
            tc.tile_pool(name="ps_feat", bufs=2, space="PSUM") as ps_feat,
            tc.tile_pool(name="ps_s", bufs=2, space="PSUM") as ps_s,
            tc.tile_pool(name="ps_vals", bufs=1, space="PSUM") as ps_vals,
            tc.tile_pool(name="dram", bufs=1, space="DRAM") as dram,
        ):
            # ---- DRAM scratch ------------------------------------------
            feats_dram = dram.tile([NCH, 128, H * O], bf16, tag="featsd", name="featsd")
            # last stripe's d goes in two halves (jt 0-3 / 4-7) to shorten
            # the end-of-kernel drain; cols are laid out jt*4+h so halves
            # are contiguous.
            d_in = [dram.tile([128, 32], f32, tag=f"din{s}", name=f"din{s}")
                    for s in range(N_STRIPES)]
            d_out = [dram.tile([128, 32], f32, tag=f"dout{s}", name=f"dout{s}")
                     for s in range(N_STRIPES)]
            dh_in = [dram.tile([128, 16], f32, tag=f"dhin{p}", name=f"dhin{p}")
                     for p in range(2)]
            dh_out = [dram.tile([128, 16], f32, tag=f"dhout{p}", name=f"dhout{p}")
                      for p in range(2)]
            st_in = dram.tile([1, 32], f32, tag="stin", name="stin")
            st_out = dram.tile([1, 32], f32, tag="stout", name="stout")
            dum_in = dram.tile([128, 32], f32, tag="dumin", name="dumin")
            dum_out = dram.tile([128, 32], f32, tag="dumout", name="dumout")
            arow_dram = dram.tile([H, MY_N], f32, tag="arowd", name="arowd")

            # warm up the collective stack before ANY other DMA is queued
            # (one-time comm init + first-transfer latency ~100us otherwise
            # lands on stripe 0's d AllReduce; its data DMA must not sit
            # behind bulk mask/feats descriptors)
            zr = constp.tile([128, 32], f32, tag="zr", name="zr")
            nc.vector.memset(zr[:], 0.0)
            nc.sync.dma_start(dum_in[:], zr[:])
            nc.gpsimd.collective_compute(
                "AllReduce", Alu.add, replica_groups=RG,
                ins=[dum_in.opt()], outs=[dum_out.opt()])

            # ---- constants ---------------------------------------------
            identf = constp.tile([128, 128], f32, tag="identf", name="identf")
            nc.sync.dma_start(identf[:], identf_in)
            ones_col = constp.tile([128, 1], f32, tag="ones_col", name="ones_col")
            nc.vector.memset(ones_col[:], 1.0)
            ones_row = constp.tile([1, 512], f32, tag="ones_row", name="ones_row")
            nc.vector.memset(ones_row[:], 1.0)
            sel39 = constp.tile([128, 1], f32, tag="sel39", name="sel39")
            nc.sync.dma_start(sel39[:], sel39_in)
            invsel39 = constp.tile([128, 1], f32, tag="invsel39", name="invsel39")
            nc.sync.dma_start(invsel39[:], invsel39_in)

            # ---- resident SBUF inputs ----------------------------------
            rnaT = [constp.tile([128, MY_N], bf16, tag=f"rnaT{fc}", name=f"rnaT{fc}")
                    for fc in range(FC)]
            disT = [constp.tile([128, MY_N], bf16, tag=f"disT{fc}", name=f"disT{fc}")
                    for fc in range(FC)]
            for fc in range(FC):
                nc.sync.dma_start(rnaT[fc][:], in_rnaT[fc])
                nc.sync.dma_start(disT[fc][:], in_disT[fc])
            projb = {}
            for tname, pap in (("rna", projc_rna), ("dis", projc_dis)):
                for fc in range(FC):
                    pb = constp.tile([128, H * O], bf16, tag=f"pb_{tname}{fc}",
                                     name=f"pb_{tname}{fc}")
                    nc.sync.dma_start(pb[:], pap[fc])
                    projb[(tname, fc)] = pb
            wrT = []
            for fc in range(FC):
                wt = constp.tile([128, O], bf16, tag=f"wrT{fc}", name=f"wrT{fc}")
                nc.sync.dma_start(wt[:], wrT_in[fc])
                wrT.append(wt)

            # ---- score vectors -> q[type][fc] = [128f, 8] bf16 ----------
            # cols 0..3 = src head h, 4..7 = tgt head h
            q_rhs = {(t, fc): constp.tile([128, 8], bf16, tag=f"q{t}{fc}", name=f"q{t}{fc}")
                     for t in ("rna", "dis") for fc in range(FC)}
            for si, sap in ((0, score_src), (1, score_tgt)):
                for h in range(H):
                    scol = pro.tile([128, 1], f32, tag="scol", name="scol", bufs=2)
                    nc.sync.dma_start(scol[:], sap[h])
                    tpq = ps_s.tile([128, 128], f32, tag="tpq", name="tpq")
                    nc.tensor.transpose(tpq[0:1, :], scol[:], identf[:])
                    wrow = pro.tile([1, 128], f32, tag="wrow", name="wrow", bufs=2)
                    nc.vector.tensor_copy(wrow[:], tpq[0:1, :])
                    wb = pro.tile([128, 128], f32, tag="wb", name="wb", bufs=2)
                    nc.gpsimd.partition_broadcast(wb[:], wrow[:])
                    for tname in ("rna", "dis"):
                        for fc in range(FC):
                            qcol = pro.tile([128, 1], f32, tag="qcol", name="qcol", bufs=2)
                            qscr = pro.tile([128, O], f32, tag="qscr", name="qscr", bufs=2)
                            nc.vector.scalar_tensor_tensor(
                                qscr[:], projb[(tname, fc)][:, h * 128:(h + 1) * 128],
                                1.0, wb[:],
                                op0=Alu.mult, op1=Alu.mult, accum_out=qcol[:])
                            nc.vector.tensor_copy(
                                q_rhs[(tname, fc)][:, si * 4 + h:si * 4 + h + 1], qcol[:])

            # ---- s for my rows -> A_bcast[h] = [128, MY_N] bf16 ---------
            for ic in range(MY_N // 128):
                ps_sr = ps_s.tile([128, 8], f32, tag="small", name="pssr")
                k = 0
                for tname, Tt in (("rna", rnaT), ("dis", disT)):
                    for fc in range(FC):
                        nc.tensor.matmul(ps_sr[:], Tt[fc][:, ic * 128:(ic + 1) * 128],
                                         q_rhs[(tname, fc)][:],
                                         start=(k == 0), stop=(k == 3))
                        k += 1
                srow = pro.tile([128, 8], f32, tag="srow", name="srow", bufs=2)
                nc.vector.tensor_copy(srow[:], ps_sr[:])
                tps = ps_s.tile([128, 128], f32, tag="tpq", name="tps")
                nc.tensor.transpose(tps[0:8, :], srow[:], identf[:])
                srT = pro.tile([8, 128], f32, tag="srT", name="srT", bufs=2)
                nc.vector.tensor_copy(srT[:], tps[0:8, :])
                for h in range(H):
                    nc.sync.dma_start(arow_dram[h, ic * 128:(ic + 1) * 128], srT[h:h + 1, :])
            A_bcast = []
            for h in range(H):
                af = pro.tile([128, MY_N], f32, tag="af", name="af", bufs=1)
                nc.sync.dma_start(af[:], arow_dram[h:h + 1, :].partition_broadcast(128))
                ab = constp.tile([128, MY_N], bf16, tag=f"ab{h}", name=f"ab{h}")
                nc.vector.tensor_copy(ab[:], af[:])
                A_bcast.append(ab)

            # ---- full feats + s_all (shared inT chunk loop, PE work) ----
            # feats[ch] spilled to DRAM bf16; s_all[ch] kept in SBUF. The
            # chunk loop is spread through the stripe loop (stripe s emits
            # the chunks stripe s+2 will need) so the DVE/ACT queues reach
            # stripe 0's elementwise work early.
            s_all = [constp.tile([128, 8], f32, tag=f"sall{ch}", name=f"sall{ch}")
                     for ch in range(NCH)]

            def chunk_parts(ch):
                if ch < SPLIT_CH:
                    return ["rna"]
                if ch > SPLIT_CH:
                    return ["dis"]
                return ["rna", "dis"]

            def emit_chunk(ch):
                inTs = []
                for fc in range(FC):
                    itl = pro.tile([128, 128], bf16, tag="inT_ld", name="inT_ld", bufs=6)
                    nc.sync.dma_start(itl[:], inT[fc, :, ch * 128:(ch + 1) * 128])
                    inTs.append(itl)
                tmpf, tmps = {}, {}
                for tname in chunk_parts(ch):
                    ps_f = ps_feat.tile([128, H * O], f32, tag="psf", name="psf")
                    ps_sc = ps_s.tile([128, 8], f32, tag="small", name="pssc")
                    for fc in range(FC):
                        nc.tensor.matmul(ps_f[:], inTs[fc][:], projb[(tname, fc)][:],
                                         start=(fc == 0), stop=(fc == FC - 1))
                    for fc in range(FC):
                        nc.tensor.matmul(ps_sc[:], inTs[fc][:], q_rhs[(tname, fc)][:],
                                         start=(fc == 0), stop=(fc == FC - 1))
                    tmpf[tname] = ps_f
                    tmps[tname] = ps_sc
                if ch != SPLIT_CH:
                    tname = chunk_parts(ch)[0]
                    fsb = pro.tile([128, H * O], bf16, tag="fsb", name="fsb", bufs=4)
                    if ch % 2 == 0:
                        nc.vector.tensor_copy(fsb[:], tmpf[tname][:])
                    else:
                        nc.scalar.activation(fsb[:], tmpf[tname][:], Act.Copy)
                    nc.sync.dma_start(feats_dram[ch], fsb[:])
                    nc.vector.tensor_copy(s_all[ch][:], tmps[tname][:])
                else:
                    # row-wise blend: rows < SPLIT_ROW take rna, rest take dis
                    t1f = pro.tile([128, H * O], f32, tag="blendf", name="blendf", bufs=1)
                    nc.vector.tensor_scalar_mul(t1f[:], tmpf["dis"][:], invsel39[:])
                    fb = pro.tile([128, H * O], bf16, tag="fb39", name="fb39", bufs=1)
                    nc.vector.scalar_tensor_tensor(
                        fb[:], tmpf["rna"][:], sel39[:], t1f[:],
                        op0=Alu.mult, op1=Alu.add)
                    nc.sync.dma_start(feats_dram[ch], fb[:])
                    t1s = pro.tile([128, 8], f32, tag="blends", name="blends", bufs=1)
                    nc.vector.tensor_scalar_mul(t1s[:], tmps["dis"][:], invsel39[:])
                    nc.vector.scalar_tensor_tensor(
                        s_all[ch][:], tmps["rna"][:], sel39[:], t1s[:],
                        op0=Alu.mult, op1=Alu.add)

            # ---- main loop over j-stripes (software-pipelined) ---------
            mpool = tc.alloc_tile_pool(name="mpool", bufs=4)
            ypool = tc.alloc_tile_pool(name="ypool", bufs=4)
            epool = tc.alloc_tile_pool(name="epool", bufs=3)
            gpool = tc.alloc_tile_pool(name="gpool", bufs=3)
            vals_ps = ps_vals.tile([128, MY_N], f32, tag="big", name="vals")
            d_alls = {}
            e_tiles = {}

            LAST = N_STRIPES - 1
            # GpSimd cannot run TensorScalarPtr ops (codegen engine check),
            # so no leaky-chain offload to it.
            GPS_TILES = set()

            def emit_e_tiles(s, jt, d_all):
                ch = s * JT + jt
                mT = mpool.tile([128, MY_N], bf16, tag="mT", name="mT")
                nc.sync.dma_start(mT[:], maskT[ch * 128:(ch + 1) * 128, :])
                for h in range(H):
                    y = ypool.tile([128, MY_N], bf16, tag="y", name="y")
                    if (jt, h) in GPS_TILES:
                        z = ypool.tile([128, MY_N], bf16, tag="z", name="z", bufs=2)
                        nc.gpsimd.scalar_tensor_tensor(
                            z[:], mT[:], s_all[ch][:, 4 + h:5 + h], A_bcast[h][:],
                            op0=Alu.add, op1=Alu.add)
                        nc.gpsimd.scalar_tensor_tensor(
                            y[:], z[:], SLOPE, z[:], op0=Alu.mult, op1=Alu.max)
                    else:
                        nc.vector._custom_dve(
                            LEAKY_OP, out=y[:], in0=mT[:], in1=A_bcast[h][:],
                            s0=s_all[ch][:, 4 + h:5 + h], imm2=SLOPE)
                    e = epool.tile([128, MY_N], bf16, tag="e", name="e", bufs=67)
                    nc.scalar.activation(e[:], y[:], Act.Exp,
                                         accum_out=d_all[:, jt * 4 + h:jt * 4 + h + 1])
                    e_tiles[(s, h, jt)] = e

            def emit_d_reduce(s, d_all, half):
                # half 0 = jt 0-3 (cols 0..15), half 1 = jt 4-7 (cols 16..31)
                lo = half * 16
                din = dh_in[half] if s == LAST else None
                if s == LAST:
                    nc.sync.dma_start(dh_in[half][:], d_all[:, lo:lo + 16])
                    nc.gpsimd.collective_compute(
                        "AllReduce", Alu.add, replica_groups=RG,
                        ins=[dh_in[half].opt()], outs=[dh_out[half].opt()])
                elif half == 1:
                    nc.sync.dma_start(d_in[s][:], d_all[:])
                    nc.gpsimd.collective_compute(
                        "AllReduce", Alu.add, replica_groups=RG,
                        ins=[d_in[s].opt()], outs=[d_out[s].opt()])

            def emit_dinv(s):
                if s == LAST:
                    out = []
                    for half in range(2):
                        d_sum = dpool.tile([128, 16], f32, tag="dsumh", name="dsumh")
                        nc.sync.dma_start(d_sum[:], dh_out[half][:])
                        dinv = dpool.tile([128, 16], f32, tag="dinvh", name="dinvh")
                        nc.vector.reciprocal(dinv[:], d_sum[:])
                        out.append((dinv, -half * 16))
                    return out
                d_sum = dpool.tile([128, 32], f32, tag="dsum", name="dsum")
                nc.sync.dma_start(d_sum[:], d_out[s][:])
                dinv = dpool.tile([128, 32], f32, tag="dinv", name="dinv")
                nc.vector.reciprocal(dinv[:], d_sum[:])
                return [(dinv, 0), (dinv, 0)]

            def emit_bmm_jt(s, jt, dinv, coff):
                ch = s * JT + jt
                fst4 = gpool.tile([128, H * O], bf16, tag="fst4", name="fst4")
                nc.sync.dma_start(fst4[:], feats_dram[ch])
                g4 = gpool.tile([128, H * O], bf16, tag="g4", name="g4")
                for h in range(H):
                    c = jt * 4 + h + coff
                    nc.vector.tensor_scalar_mul(
                        g4[:, h * 128:(h + 1) * 128], fst4[:, h * 128:(h + 1) * 128],
                        dinv[:, c:c + 1])
                for h in range(H):
                    e = e_tiles.pop((s, h, jt))
                    first = (s == 0) and h == 0 and jt == 0
                    last = (s == LAST) and h == H - 1 and jt == JT - 1
                    nc.tensor.matmul(vals_ps[:, 0:512], g4[:, h * 128:(h + 1) * 128],
                                     e[:, 0:512], start=first, stop=last)
                    nc.tensor.matmul(vals_ps[:, 512:1024], g4[:, h * 128:(h + 1) * 128],
                                     e[:, 512:1024], start=first, stop=last)

            for ch in range(2 * JT):
                emit_chunk(ch)
            dinv_prev = None
            for s in range(N_STRIPES):
                d_all = dpool.tile([128, 32], f32, tag="dall", name="dall")
                for jt in range(JT):
                    emit_e_tiles(s, jt, d_all)
                    if jt == JT // 2 - 1:
                        emit_d_reduce(s, d_all, 0)
                    elif jt == JT - 1:
                        emit_d_reduce(s, d_all, 1)
                    # interleave previous stripe's normalize+bmm into the
                    # second half of this stripe (d(s-1) has arrived by then)
                    if s >= 1:
                        if jt == 3:
                            dinv_prev = emit_dinv(s - 1)
                        elif jt >= 4:
                            dv, coff = dinv_prev[(jt - 4) // 2]
                            emit_bmm_jt(s - 1, 2 * (jt - 4), dv, coff)
                            emit_bmm_jt(s - 1, 2 * (jt - 4) + 1, dv, coff)
                if s + 2 < N_STRIPES:
                    for jt in range(JT):
                        emit_chunk((s + 2) * JT + jt)
            dinv_last = emit_dinv(LAST)
            for jt in range(JT):
                dv, coff = dinv_last[jt // 4]
                emit_bmm_jt(LAST, jt, dv, coff)

            # ---- tail: instance norm + residual + elu ------------------
            gpool.release()
            epool.release()
            ypool.release()
            mpool.release()
            tailp = tc.alloc_tile_pool(name="tail", bufs=1)
            vs = tailp.tile([128, MY_N], f32, tag="vs", name="vs")
            srow1 = tailp.tile([128, 1], f32, tag="srow1", name="srow1")
            nc.scalar.activation(vs[:], vals_ps[:], Act.Copy, scale=0.25,
                                 accum_out=srow1[:])
            vsq = tailp.tile([128, MY_N], f32, tag="vsq", name="vsq")
            srow2 = tailp.tile([128, 1], f32, tag="srow2", name="srow2")
            nc.scalar.activation(vsq[:], vs[:], Act.Square, accum_out=srow2[:])

            ps1 = ps_s.tile([1, 1], f32, tag="small", name="ps1")
            nc.tensor.matmul(ps1[:], srow1[:], ones_col[:])
            ps2 = ps_s.tile([1, 1], f32, tag="small", name="ps2")
            nc.tensor.matmul(ps2[:], srow2[:], ones_col[:])
            stv = tailp.tile([1, 32], f32, tag="stv", name="stv")
            nc.vector.memset(stv[:], 0.0)
            nc.vector.tensor_copy(stv[0:1, 0:1], ps1[:])
            nc.vector.tensor_copy(stv[0:1, 16:17], ps2[:])
            nc.sync.dma_start(st_in[:], stv[:])
            nc.gpsimd.collective_compute(
                "AllReduce", Alu.add, replica_groups=RG,
                ins=[st_in.opt()], outs=[st_out.opt()])
            str_ = tailp.tile([1, 32], f32, tag="str", name="str")
            nc.sync.dma_start(str_[:], st_out[:])

            c = 1.0 / float(N * O)
            mu = tailp.tile([1, 1], f32, tag="mu", name="mu")
            nc.vector.tensor_scalar_mul(mu[:], str_[0:1, 0:1], c)
            m2 = tailp.tile([1, 1], f32, tag="m2", name="m2")
            nc.vector.tensor_scalar_mul(m2[:], str_[0:1, 16:17], c)
            mu2 = tailp.tile([1, 1], f32, tag="mu2", name="mu2")
            nc.vector.tensor_mul(mu2[:], mu[:], mu[:])
            var = tailp.tile([1, 1], f32, tag="var", name="var")
            nc.vector.tensor_sub(var[:], m2[:], mu2[:])
            vpe = tailp.tile([1, 1], f32, tag="vpe", name="vpe")
            nc.vector.tensor_scalar_add(vpe[:], var[:], EPS)
            sd = tailp.tile([1, 1], f32, tag="sd", name="sd")
            nc.scalar.activation(sd[:], vpe[:], Act.Sqrt)
            rstd = tailp.tile([1, 1], f32, tag="rstd", name="rstd")
            nc.vector.reciprocal(rstd[:], sd[:])
            negmurs = tailp.tile([1, 1], f32, tag="negmurs", name="negmurs")
            nc.vector.tensor_mul(negmurs[:], mu[:], rstd[:])
            nc.vector.tensor_scalar_mul(negmurs[:], negmurs[:], -1.0)

            a_col = tailp.tile([128, 1], f32, tag="acol", name="acol")
            nc.gpsimd.partition_broadcast(a_col[:], rstd[:])
            b_row = tailp.tile([1, 128], f32, tag="brow", name="brow")
            nc.scalar.activation(b_row[:], ones_row[0:1, 0:128], Act.Copy,
                                 scale=negmurs[:])

            rowsT = [tailp.tile([128, MY_N], bf16, tag=f"rowsT{fc}", name=f"rowsT{fc}")
                     for fc in range(FC)]
            for fc in range(FC):
                nc.vector.tensor_add(rowsT[fc][:], rnaT[fc][:], disT[fc][:])

            r_ps = ps_vals.tile([128, MY_N], f32, tag="big", name="resid")
            for half in range(2):
                sl = slice(half * 512, (half + 1) * 512)
                for fc in range(FC):
                    nc.tensor.matmul(r_ps[:, sl], wrT[fc][:], rowsT[fc][:, sl],
                                     start=(fc == 0), stop=False)
                nc.tensor.matmul(r_ps[:, sl], b_row[:], ones_row[:],
                                 start=False, stop=True)

            pre = tailp.tile([128, MY_N], f32, tag="pre", name="pre")
            nc.vector.scalar_tensor_tensor(pre[:], vs[:], a_col[:], r_ps[:],
                                           op0=Alu.mult, op1=Alu.add)
            negp = tailp.tile([128, MY_N], f32, tag="negp", name="negp")
            nc.vector.tensor_scalar_min(negp[:], pre[:], 0.0)
            w = tailp.tile([128, MY_N], f32, tag="w", name="w")
            nc.scalar.activation(w[:], negp[:], Act.Exp)
            r1 = tailp.tile([128, MY_N], f32, tag="r1", name="r1")
            nc.vector.tensor_scalar_max(r1[:], pre[:], 0.0)
            outt = tailp.tile([128, MY_N], f32, tag="outt", name="outt")
            nc.vector.scalar_tensor_tensor(outt[:], w[:], -1.0, r1[:],
                                           op0=Alu.add, op1=Alu.add)
            nc.sync.dma_start(out_dram, outt[:])
            tailp.release()

    nc.compile()
    return nc


def _get_nc():
    if "nc" not in _cached:
        _cached["nc"] = _build()
    return _cached["nc"]


def kernel(input_mat, connectivity_mask, proj_rna, proj_dis, score_src,
           score_tgt, residual_w):
    import ml_dtypes
    from concourse.bass_utils import run_bass_kernel_spmd

    bf16 = ml_dtypes.bfloat16
    nc = _get_nc()
    input_mat = np.asarray(input_mat, np.float32)
    connectivity_mask = np.asarray(connectivity_mask, np.float32)
    proj_rna = np.asarray(proj_rna, np.float32)
    proj_dis = np.asarray(proj_dis, np.float32)
    residual_w = np.asarray(residual_w, np.float32)

    ident = np.eye(128, dtype=np.float32)
    sel39 = (np.arange(128) < SPLIT_ROW).astype(np.float32)[:, None]
    rna_mask = (np.arange(N) < N_RNA).astype(np.float32)[:, None]
    in_rna_full = input_mat * rna_mask
    in_dis_full = input_mat * (1.0 - rna_mask)

    # layout-only host prep: transposes + bf16 casts
    inT = np.ascontiguousarray(input_mat.T).reshape(FC, 128, N).astype(bf16)
    projc_rna = np.ascontiguousarray(
        np.transpose(proj_rna, (1, 0, 2)).reshape(F, H * O)
    ).reshape(FC, 128, H * O).astype(bf16)
    projc_dis = np.ascontiguousarray(
        np.transpose(proj_dis, (1, 0, 2)).reshape(F, H * O)
    ).reshape(FC, 128, H * O).astype(bf16)
    wrT = np.ascontiguousarray(residual_w.T).reshape(FC, 128, O).astype(bf16)

    in_maps = []
    for k in range(N_CORES):
        r0, r1 = k * MY_N, (k + 1) * MY_N
        in_maps.append({
            "maskT": np.ascontiguousarray(connectivity_mask[r0:r1].T).astype(bf16),
            "inT": inT,
            "in_rnaT": np.ascontiguousarray(in_rna_full[r0:r1].T).reshape(
                FC, 128, MY_N).astype(bf16),
            "in_disT": np.ascontiguousarray(in_dis_full[r0:r1].T).reshape(
                FC, 128, MY_N).astype(bf16),
            "projc_rna": projc_rna,
            "projc_dis": projc_dis,
            "score_src": np.asarray(score_src, np.float32),
            "score_tgt": np.asarray(score_tgt, np.float32),
            "wrT": wrT,
            "identf": ident,
            "sel39": sel39,
            "invsel39": 1.0 - sel39,
        })

    res = run_bass_kernel_spmd(nc, in_maps, core_ids=list(range(N_CORES)))
    _cached["last_result"] = res
    out = np.empty((N, O), np.float32)
    for k in range(N_CORES):
        out[k * MY_N:(k + 1) * MY_N, :] = res.results[k]["out"].T
    return out


# revision 35
# speedup vs baseline: 1.0737x; 1.0058x over previous
"""Trainium2 Bass kernel for nn_HGraphAttentionLayer (GAT-style layer, 8 NeuronCores).

Math (reference):
  feats[h,n,o]  = concat(input[:5000] @ proj_rna[h], input[5000:] @ proj_dis[h])
  s_src[h,n]    = feats[h,n,:] @ score_src[h];  s_tgt likewise
  attn[h,i,j]   = softmax_over_i( mask[i,j] + leaky_relu(s_src[h,i]+s_tgt[h,j], 0.2) )
  vals[i,o]     = mean_h( sum_j attn[h,i,j] * feats[h,j,o] )
  out           = elu( instancenorm(vals) + input @ residual_w.T )

Sharding: each of the 8 cores owns N/8 = 1024 query rows (i). The mask arrives
host-transposed to [j, i] bf16 so each e-tile [128j, 1024i] is a straight DMA.
A custom fused DVE op computes y = leaky(mask + a_i + b_j) in one 1-cyc/elem
pass; ACT computes e = exp(y) with the d-partial as free accumulation. d is
completed with one small AllReduce per j-stripe; the bmm contracts e against
feats/d with vals^T accumulating in PSUM across all stripes.
"""
import numpy as np

N, F, H, O = 8192, 256, 4, 128
N_CORES = 8
MY_N = N // N_CORES          # 1024 rows per core
N_RNA = 5000
SLOPE = 0.2
EPS = 1e-5
N_STRIPES = 8
SJ = N // N_STRIPES          # 1024 j per stripe
JT = SJ // 128               # 8 j-tiles per stripe
NCH = N // 128               # 64 n-chunks
FC = F // 128                # 2 f-chunks
SPLIT_CH = N_RNA // 128      # chunk 39 contains the rna/dis boundary
SPLIT_ROW = N_RNA - SPLIT_CH * 128  # row 8 within chunk 39

_cached = {}


def _leaky_2x_uop():
    """Hand-written 2X_1PORT micro-op: both packed bf16 elements of each
    32-bit port read are computed per cycle (element A in blocks 0-3,
    element B in blocks 4-7; A's result rides delay lane 0 to the packed
    write). Mirrors the stock TensorTensor 2x program's structure."""
    from concourse.dve_uop import (
        UopConfig, UopDpConfig, InpSel, OutSel, AluOp, AluInp, DelayInp,
        OutPath, Trigger,
    )

    PD = DelayInp.PREV_DELAY if hasattr(DelayInp, "PREV_DELAY") else DelayInp(5)
    PA = DelayInp.PREV_ALU_OUT

    def blk(op, s0, s1, delay_sel, delay_en):
        d = [PD] * 7
        for i, v in delay_sel.items():
            d[i] = v
        en = [0] * 7
        for i in delay_en:
            en[i] = 1
        return UopDpConfig(op=op, alu_src0=AluInp(s0), alu_src1=AluInp(s1),
                           delay=d, alu_out_enable=1, delay_enable=en)

    A = AluInp
    dp = [
        # A: s = src0 + src1
        blk(AluOp.ADD, A.PREV_DELAY_0, A.PREV_DELAY_1, {}, [2, 3, 4, 5]),
        # A: t = s + C0(b)
        blk(AluOp.ADD, A.PREV_ALU_OUT, A.PREV_DELAY_4, {}, [2, 3, 4, 5]),
        # A: u = t * C2(0.2); lane0 <- t
        blk(AluOp.MULTIPLY, A.PREV_ALU_OUT, A.PREV_DELAY_5,
            {0: PA}, [0, 2, 3, 4, 5]),
        # A: y = max(u, t)
        blk(AluOp.MAX, A.PREV_ALU_OUT, A.PREV_DELAY_0, {}, [2, 3, 4, 5]),
        # B: s = src0_hi + src1_hi; lane0 <- yA
        blk(AluOp.ADD, A.PREV_DELAY_2, A.PREV_DELAY_3, {0: PA}, [0, 4, 5]),
        # B: t = s + C0
        blk(AluOp.ADD, A.PREV_ALU_OUT, A.PREV_DELAY_4, {}, [0, 5]),
        # B: u = t * C2; lane1 <- t
        blk(AluOp.MULTIPLY, A.PREV_ALU_OUT, A.PREV_DELAY_5, {1: PA}, [0, 1]),
        # B: y = max(u, t)
        blk(AluOp.MAX, A.PREV_ALU_OUT, A.PREV_DELAY_1, {}, [0]),
    ]
    return UopConfig(
        inp=[InpSel.ZERO, InpSel.SRC_0, InpSel.SRC_1, InpSel.SRC_0_HI,
             InpSel.SRC_1_HI, InpSel.CONST_0, InpSel.CONST_2, InpSel.ZERO],
        inp_enable=[0, 1, 1, 1, 1, 1, 1, 0],
        out={OutPath.WR0_LO: OutSel.DELAY_0, OutPath.WR0_HI: OutSel.ALU_OUT,
             OutPath.WR1_LO: OutSel.ALU_OUT, OutPath.WR1_HI: OutSel.ALU_OUT},
        out_enable={OutPath.WR0_LO: 1, OutPath.WR0_HI: 1,
                    OutPath.WR1_LO: 0, OutPath.WR1_HI: 0},
        require_inp0=1,
        require_inp1=1,
        trigger=(Trigger.SRC_TENSOR_DONE, Trigger.NONE, Trigger.NONE),
        datapath_config=dp,
    )


def _register_masked_leaky():
    """Runtime-register the fused DVE op  y = max(t, 0.2*t), t = in0+in1+s0.
    in0 = mask^T tile, in1 = a_i broadcast, s0 = b_j per-partition column.
    A hand-written 2X_1PORT variant doubles bf16 throughput."""
    from concourse import dve_ops
    from concourse.dve_ops import DveOp, OPS
    from concourse.dve_spec import Spec, Src0, Src1, C0, C2, maxx, lower
    from concourse.dve_uop import DveOpSpec
    from concourse.dve_table_gen import dve_ver_for

    for op in OPS:
        if op.name == "MASKED_LEAKY_ATTN":
            return op

    t = Src0 + Src1 + C0
    spec = Spec(
        body=maxx(t, t * C2),
        reference=lambda in0, in1, s0, s1, imm2: (
            lambda tt: np.maximum(tt, tt * imm2)
        )(np.asarray(in0, np.float32) + np.asarray(in1, np.float32) + s0),
    )
    op = DveOp("MASKED_LEAKY_ATTN", spec, subdim=False, uops_sha={})
    OPS.append(op)
    row = dve_ops._CUSTOM_DVE_ROW_BASE + len(OPS) - 1
    assert row < 0x20
    dve_ops._SUB_OPCODE_FOR_NAME[op.name] = row
    dve_ops.CUSTOM_DVE_SPECS[op.name] = spec
    for trn in ("TRN2",):
        ver = dve_ver_for(trn)
        lowered = DveOpSpec(
            name=op.name,
            opcode=row,
            uops=lower(spec, ver=ver),
            uops_2x=[_leaky_2x_uop()],
            perf_max=1,
            rd1_en=True,
        )
        lowered.validate(ver)
        op.uops_sha[ver] = lowered.sha(ver)
        dve_ops._COMPILE_CACHE[(op.name, ver)] = lowered
    return op


def _build():
    import concourse.bass as bass
    import concourse.bacc as bacc
    import concourse.mybir as mybir
    import concourse.tile as tile

    f32 = mybir.dt.float32
    bf16 = mybir.dt.bfloat16
    Alu = mybir.AluOpType
    Act = mybir.ActivationFunctionType

    LEAKY_OP = _register_masked_leaky()

    nc = bacc.Bacc("TRN2", target_bir_lowering=False, debug=False,
                   enable_asserts=False, num_devices=N_CORES)

    # ---- I/O (host pre-transposed / bf16 where layout-only) -------------
    maskT = nc.dram_tensor("maskT", [N, MY_N], bf16, kind="ExternalInput").ap()
    inT = nc.dram_tensor("inT", [FC, 128, N], bf16, kind="ExternalInput").ap()
    in_rnaT = nc.dram_tensor("in_rnaT", [FC, 128, MY_N], bf16, kind="ExternalInput").ap()
    in_disT = nc.dram_tensor("in_disT", [FC, 128, MY_N], bf16, kind="ExternalInput").ap()
    projc_rna = nc.dram_tensor("projc_rna", [FC, 128, H * O], bf16, kind="ExternalInput").ap()
    projc_dis = nc.dram_tensor("projc_dis", [FC, 128, H * O], bf16, kind="ExternalInput").ap()
    score_src = nc.dram_tensor("score_src", [H, O, 1], f32, kind="ExternalInput").ap()
    score_tgt = nc.dram_tensor("score_tgt", [H, O, 1], f32, kind="ExternalInput").ap()
    wrT_in = nc.dram_tensor("wrT", [FC, 128, O], bf16, kind="ExternalInput").ap()
    identf_in = nc.dram_tensor("identf", [128, 128], f32, kind="ExternalInput").ap()
    sel39_in = nc.dram_tensor("sel39", [128, 1], f32, kind="ExternalInput").ap()
    invsel39_in = nc.dram_tensor("invsel39", [128, 1], f32, kind="ExternalInput").ap()
    out_dram = nc.dram_tensor("out", [O, MY_N], f32, kind="ExternalOutput").ap()

    RG = [list(range(N_CORES))]

    with tile.TileContext(nc) as tc:
        with (
            tc.tile_pool(name="const", bufs=1) as constp,
            tc.tile_pool(name="pro", bufs=3) as pro,
            tc.tile_pool(name="dpool", bufs=3) as dpool,
            tc.tile_pool(name="ps_feat", bufs=2, space="PSUM") as ps_feat,
            tc.tile_pool(name="ps_s", bufs=2, space="PSUM") as ps_s,
            tc.tile_pool(name="ps_vals", bufs=1, space="PSUM") as ps_vals,
            tc.tile_pool(name="dram", bufs=1, space="DRAM") as dram,
        ):
            # ---- DRAM scratch ------------------------------------------
            feats_dram = dram.tile([NCH, 128, H * O], bf16, tag="featsd", name="featsd")
            # last stripe's d goes in two halves (jt 0-3 / 4-7) to shorten
            # the end-of-kernel drain; cols are laid out jt*4+h so halves
            # are contiguous.
            d_in = [dram.tile([128, 32], f32, tag=f"din{s}", name=f"din{s}")
                    for s in range(N_STRIPES)]
            d_out = [dram.tile([128, 32], f32, tag=f"dout{s}", name=f"dout{s}")
                     for s in range(N_STRIPES)]
            dh_in = [dram.tile([128, 8], f32, tag=f"dhin{p}", name=f"dhin{p}")
                     for p in range(4)]
            dh_out = [dram.tile([128, 8], f32, tag=f"dhout{p}", name=f"dhout{p}")
                      for p in range(4)]
            st_in = dram.tile([1, 32], f32, tag="stin", name="stin")
            st_out = dram.tile([1, 32], f32, tag="stout", name="stout")
            dum_in = dram.tile([128, 32], f32, tag="dumin", name="dumin")
            dum_out = dram.tile([128, 32], f32, tag="dumout", name="dumout")
            arow_dram = dram.tile([H, MY_N], f32, tag="arowd", name="arowd")

            # warm up the collective stack before ANY other DMA is queued
            # (one-time comm init + first-transfer latency ~100us otherwise
            # lands on stripe 0's d AllReduce; its data DMA must not sit
            # behind bulk mask/feats descriptors)
            zr = constp.tile([128, 32], f32, tag="zr", name="zr")
            nc.vector.memset(zr[:], 0.0)
            nc.sync.dma_start(dum_in[:], zr[:])
            nc.gpsimd.collective_compute(
                "AllReduce", Alu.add, replica_groups=RG,
                ins=[dum_in.opt()], outs=[dum_out.opt()])

            # ---- constants ---------------------------------------------
            identf = constp.tile([128, 128], f32, tag="identf", name="identf")
            nc.sync.dma_start(identf[:], identf_in)
            ones_col = constp.tile([128, 1], f32, tag="ones_col", name="ones_col")
            nc.vector.memset(ones_col[:], 1.0)
            ones_row = constp.tile([1, 512], f32, tag="ones_row", name="ones_row")
            nc.vector.memset(ones_row[:], 1.0)
            sel39 = constp.tile([128, 1], f32, tag="sel39", name="sel39")
            nc.sync.dma_start(sel39[:], sel39_in)
            invsel39 = constp.tile([128, 1], f32, tag="invsel39", name="invsel39")
            nc.sync.dma_start(invsel39[:], invsel39_in)

            # ---- resident SBUF inputs ----------------------------------
            rnaT = [constp.tile([128, MY_N], bf16, tag=f"rnaT{fc}", name=f"rnaT{fc}")
                    for fc in range(FC)]
            disT = [constp.tile([128, MY_N], bf16, tag=f"disT{fc}", name=f"disT{fc}")
                    for fc in range(FC)]
            for fc in range(FC):
                nc.sync.dma_start(rnaT[fc][:], in_rnaT[fc])
                nc.sync.dma_start(disT[fc][:], in_disT[fc])
            projb = {}
            for tname, pap in (("rna", projc_rna), ("dis", projc_dis)):
                for fc in range(FC):
                    pb = constp.tile([128, H * O], bf16, tag=f"pb_{tname}{fc}",
                                     name=f"pb_{tname}{fc}")
                    nc.sync.dma_start(pb[:], pap[fc])
                    projb[(tname, fc)] = pb
            wrT = []
            for fc in range(FC):
                wt = constp.tile([128, O], bf16, tag=f"wrT{fc}", name=f"wrT{fc}")
                nc.sync.dma_start(wt[:], wrT_in[fc])
                wrT.append(wt)

            # ---- score vectors -> q[type][fc] = [128f, 8] bf16 ----------
            # cols 0..3 = src head h, 4..7 = tgt head h
            q_rhs = {(t, fc): constp.tile([128, 8], bf16, tag=f"q{t}{fc}", name=f"q{t}{fc}")
                     for t in ("rna", "dis") for fc in range(FC)}
            for si, sap in ((0, score_src), (1, score_tgt)):
                for h in range(H):
                    scol = pro.tile([128, 1], f32, tag="scol", name="scol", bufs=2)
                    nc.sync.dma_start(scol[:], sap[h])
                    tpq = ps_s.tile([128, 128], f32, tag="tpq", name="tpq")
                    nc.tensor.transpose(tpq[0:1, :], scol[:], identf[:])
                    wrow = pro.tile([1, 128], f32, tag="wrow", name="wrow", bufs=2)
                    nc.vector.tensor_copy(wrow[:], tpq[0:1, :])
                    wb = pro.tile([128, 128], f32, tag="wb", name="wb", bufs=2)
                    nc.gpsimd.partition_broadcast(wb[:], wrow[:])
                    for tname in ("rna", "dis"):
                        for fc in range(FC):
                            qcol = pro.tile([128, 1], f32, tag="qcol", name="qcol", bufs=2)
                            qscr = pro.tile([128, O], f32, tag="qscr", name="qscr", bufs=2)
                            nc.vector.scalar_tensor_tensor(
                                qscr[:], projb[(tname, fc)][:, h * 128:(h + 1) * 128],
                                1.0, wb[:],
                                op0=Alu.mult, op1=Alu.mult, accum_out=qcol[:])
                            nc.vector.tensor_copy(
                                q_rhs[(tname, fc)][:, si * 4 + h:si * 4 + h + 1], qcol[:])

            # ---- s for my rows -> A_bcast[h] = [128, MY_N] bf16 ---------
            for ic in range(MY_N // 128):
                ps_sr = ps_s.tile([128, 8], f32, tag="small", name="pssr")
                k = 0
                for tname, Tt in (("rna", rnaT), ("dis", disT)):
                    for fc in range(FC):
                        nc.tensor.matmul(ps_sr[:], Tt[fc][:, ic * 128:(ic + 1) * 128],
                                         q_rhs[(tname, fc)][:],
                                         start=(k == 0), stop=(k == 3))
                        k += 1
                srow = pro.tile([128, 8], f32, tag="srow", name="srow", bufs=2)
                nc.vector.tensor_copy(srow[:], ps_sr[:])
                tps = ps_s.tile([128, 128], f32, tag="tpq", name="tps")
                nc.tensor.transpose(tps[0:8, :], srow[:], identf[:])
                srT = pro.tile([8, 128], f32, tag="srT", name="srT", bufs=2)
                nc.vector.tensor_copy(srT[:], tps[0:8, :])
                for h in range(H):
                    nc.sync.dma_start(arow_dram[h, ic * 128:(ic + 1) * 128], srT[h:h + 1, :])
            A_bcast = []
            for h in range(H):
                af = pro.tile([128, MY_N], f32, tag="af", name="af", bufs=1)
                nc.sync.dma_start(af[:], arow_dram[h:h + 1, :].partition_broadcast(128))
                ab = constp.tile([128, MY_N], bf16, tag=f"ab{h}", name=f"ab{h}")
                nc.vector.tensor_copy(ab[:], af[:])
                A_bcast.append(ab)

            # ---- full feats + s_all (shared inT chunk loop, PE work) ----
            # feats[ch] spilled to DRAM bf16; s_all[ch] kept in SBUF. The
            # chunk loop is spread through the stripe loop (stripe s emits
            # the chunks stripe s+2 will need) so the DVE/ACT queues reach
            # stripe 0's elementwise work early.
            s_all = [constp.tile([128, 8], f32, tag=f"sall{ch}", name=f"sall{ch}")
                     for ch in range(NCH)]

            def chunk_parts(ch):
                if ch < SPLIT_CH:
                    return ["rna"]
                if ch > SPLIT_CH:
                    return ["dis"]
                return ["rna", "dis"]

            def emit_chunk(ch):
                inTs = []
                for fc in range(FC):
                    itl = pro.tile([128, 128], bf16, tag="inT_ld", name="inT_ld", bufs=6)
                    nc.sync.dma_start(itl[:], inT[fc, :, ch * 128:(ch + 1) * 128])
                    inTs.append(itl)
                tmpf, tmps = {}, {}
                for tname in chunk_parts(ch):
                    ps_f = ps_feat.tile([128, H * O], f32, tag="psf", name="psf")
                    ps_sc = ps_s.tile([128, 8], f32, tag="small", name="pssc")
                    for fc in range(FC):
                        nc.tensor.matmul(ps_f[:], inTs[fc][:], projb[(tname, fc)][:],
                                         start=(fc == 0), stop=(fc == FC - 1))
                    for fc in range(FC):
                        nc.tensor.matmul(ps_sc[:], inTs[fc][:], q_rhs[(tname, fc)][:],
                                         start=(fc == 0), stop=(fc == FC - 1))
                    tmpf[tname] = ps_f
                    tmps[tname] = ps_sc
                if ch != SPLIT_CH:
                    tname = chunk_parts(ch)[0]
                    fsb = pro.tile([128, H * O], bf16, tag="fsb", name="fsb", bufs=4)
                    nc.scalar.activation(fsb[:], tmpf[tname][:], Act.Copy)
                    nc.sync.dma_start(feats_dram[ch], fsb[:])
                    nc.vector.tensor_copy(s_all[ch][:], tmps[tname][:])
                else:
                    # row-wise blend: rows < SPLIT_ROW take rna, rest take dis
                    t1f = pro.tile([128, H * O], f32, tag="blendf", name="blendf", bufs=1)
                    nc.vector.tensor_scalar_mul(t1f[:], tmpf["dis"][:], invsel39[:])
                    fb = pro.tile([128, H * O], bf16, tag="fb39", name="fb39", bufs=1)
                    nc.vector.scalar_tensor_tensor(
                        fb[:], tmpf["rna"][:], sel39[:], t1f[:],
                        op0=Alu.mult, op1=Alu.add)
                    nc.sync.dma_start(feats_dram[ch], fb[:])
                    t1s = pro.tile([128, 8], f32, tag="blends", name="blends", bufs=1)
                    nc.vector.tensor_scalar_mul(t1s[:], tmps["dis"][:], invsel39[:])
                    nc.vector.scalar_tensor_tensor(
                        s_all[ch][:], tmps["rna"][:], sel39[:], t1s[:],
                        op0=Alu.mult, op1=Alu.add)

            # ---- main loop over j-stripes (software-pipelined) ---------
            mpool = tc.alloc_tile_pool(name="mpool", bufs=4)
            ypool = tc.alloc_tile_pool(name="ypool", bufs=4)
            epool = tc.alloc_tile_pool(name="epool", bufs=3)
            gpool = tc.alloc_tile_pool(name="gpool", bufs=3)
            vals_ps = ps_vals.tile([128, MY_N], f32, tag="big", name="vals")
            d_alls = {}
            e_tiles = {}

            LAST = N_STRIPES - 1
            # GpSimd cannot run TensorScalarPtr ops (codegen engine check),
            # so no leaky-chain offload to it.
            GPS_TILES = set()

            def emit_e_tiles(s, jt, d_all):
                ch = s * JT + jt
                mT = mpool.tile([128, MY_N], bf16, tag="mT", name="mT")
                nc.sync.dma_start(mT[:], maskT[ch * 128:(ch + 1) * 128, :])
                for h in range(H):
                    y = ypool.tile([128, MY_N], bf16, tag="y", name="y")
                    if (jt, h) in GPS_TILES:
                        z = ypool.tile([128, MY_N], bf16, tag="z", name="z", bufs=2)
                        nc.gpsimd.scalar_tensor_tensor(
                            z[:], mT[:], s_all[ch][:, 4 + h:5 + h], A_bcast[h][:],
                            op0=Alu.add, op1=Alu.add)
                        nc.gpsimd.scalar_tensor_tensor(
                            y[:], z[:], SLOPE, z[:], op0=Alu.mult, op1=Alu.max)
                    else:
                        nc.vector._custom_dve(
                            LEAKY_OP, out=y[:], in0=mT[:], in1=A_bcast[h][:],
                            s0=s_all[ch][:, 4 + h:5 + h], imm2=SLOPE)
                    e = epool.tile([128, MY_N], bf16, tag="e", name="e", bufs=67)
                    nc.scalar.activation(e[:], y[:], Act.Exp,
                                         accum_out=d_all[:, jt * 4 + h:jt * 4 + h + 1])
                    e_tiles[(s, h, jt)] = e

            def emit_d_reduce_quarter(d_all, q):
                # quarter q = jt 2q..2q+1 (cols 8q..8q+8), last stripe only
                nc.sync.dma_start(dh_in[q][:], d_all[:, q * 8:q * 8 + 8])
                nc.gpsimd.collective_compute(
                    "AllReduce", Alu.add, replica_groups=RG,
                    ins=[dh_in[q].opt()], outs=[dh_out[q].opt()])

            def emit_d_reduce(s, d_all):
                nc.sync.dma_start(d_in[s][:], d_all[:])
                nc.gpsimd.collective_compute(
                    "AllReduce", Alu.add, replica_groups=RG,
                    ins=[d_in[s].opt()], outs=[d_out[s].opt()])

            def emit_dinv(s):
                d_sum = dpool.tile([128, 32], f32, tag="dsum", name="dsum")
                nc.sync.dma_start(d_sum[:], d_out[s][:])
                dinv = dpool.tile([128, 32], f32, tag="dinv", name="dinv")
                nc.vector.reciprocal(dinv[:], d_sum[:])
                return [(dinv, 0), (dinv, 0)]

            def emit_dinv_quarter(q):
                d_sum = dpool.tile([128, 8], f32, tag="dsumq", name="dsumq")
                nc.sync.dma_start(d_sum[:], dh_out[q][:])
                dinv = dpool.tile([128, 8], f32, tag="dinvq", name="dinvq")
                nc.vector.reciprocal(dinv[:], d_sum[:])
                return dinv

            def emit_bmm_jt(s, jt, dinv, coff):
                ch = s * JT + jt
                fst4 = gpool.tile([128, H * O], bf16, tag="fst4", name="fst4")
                nc.sync.dma_start(fst4[:], feats_dram[ch])
                g4 = gpool.tile([128, H * O], bf16, tag="g4", name="g4")
                for h in range(H):
                    c = jt * 4 + h + coff
                    nc.vector.tensor_scalar_mul(
                        g4[:, h * 128:(h + 1) * 128], fst4[:, h * 128:(h + 1) * 128],
                        dinv[:, c:c + 1])
                for h in range(H):
                    e = e_tiles.pop((s, h, jt))
                    first = (s == 0) and h == 0 and jt == 0
                    last = (s == LAST) and h == H - 1 and jt == JT - 1
                    nc.tensor.matmul(vals_ps[:, 0:512], g4[:, h * 128:(h + 1) * 128],
                                     e[:, 0:512], start=first, stop=last)
                    nc.tensor.matmul(vals_ps[:, 512:1024], g4[:, h * 128:(h + 1) * 128],
                                     e[:, 512:1024], start=first, stop=last)

            for ch in range(2 * JT):
                emit_chunk(ch)
            dinv_prev = None
            for s in range(N_STRIPES):
                d_all = dpool.tile([128, 32], f32, tag="dall", name="dall")
                last = (s == LAST)
                for jt in range(JT):
                    emit_e_tiles(s, jt, d_all)
                    if last:
                        # quarter-grain d so this stripe's own bmm can start
                        # while its tail e-tiles are still being computed
                        if jt % 2 == 1:
                            emit_d_reduce_quarter(d_all, jt // 2)
                    elif jt == JT - 1:
                        emit_d_reduce(s, d_all)
                    # interleave previous stripe's normalize+bmm into the
                    # second half of this stripe (d(s-1) has arrived by then)
                    if s >= 1:
                        if jt == 3:
                            dinv_prev = emit_dinv(s - 1)
                        elif jt >= 4:
                            dv, coff = dinv_prev[(jt - 4) // 2]
                            emit_bmm_jt(s - 1, 2 * (jt - 4), dv, coff)
                            emit_bmm_jt(s - 1, 2 * (jt - 4) + 1, dv, coff)
                    if last and jt >= 5:
                        q = jt - 5
                        dvq = emit_dinv_quarter(q)
                        emit_bmm_jt(LAST, 2 * q, dvq, -8 * q)
                        emit_bmm_jt(LAST, 2 * q + 1, dvq, -8 * q)
                if s + 2 < N_STRIPES:
                    for jt in range(JT):
                        emit_chunk((s + 2) * JT + jt)
            dvq = emit_dinv_quarter(3)
            emit_bmm_jt(LAST, 6, dvq, -24)
            emit_bmm_jt(LAST, 7, dvq, -24)

            # ---- tail: instance norm + residual + elu ------------------
            gpool.release()
            epool.release()
            ypool.release()
            mpool.release()
            tailp = tc.alloc_tile_pool(name="tail", bufs=1)
            vs = tailp.tile([128, MY_N], f32, tag="vs", name="vs")
            srow1 = tailp.tile([128, 1], f32, tag="srow1", name="srow1")
            nc.scalar.activation(vs[:], vals_ps[:], Act.Copy, scale=0.25,
                                 accum_out=srow1[:])
            vsq = tailp.tile([128, MY_N], f32, tag="vsq", name="vsq")
            srow2 = tailp.tile([128, 1], f32, tag="srow2", name="srow2")
            nc.scalar.activation(vsq[:], vs[:], Act.Square, accum_out=srow2[:])

            ps1 = ps_s.tile([1, 1], f32, tag="small", name="ps1")
            nc.tensor.matmul(ps1[:], srow1[:], ones_col[:])
            ps2 = ps_s.tile([1, 1], f32, tag="small", name="ps2")
            nc.tensor.matmul(ps2[:], srow2[:], ones_col[:])
            stv = tailp.tile([1, 32], f32, tag="stv", name="stv")
            nc.vector.memset(stv[:], 0.0)
            nc.vector.tensor_copy(stv[0:1, 0:1], ps1[:])
            nc.vector.tensor_copy(stv[0:1, 16:17], ps2[:])
            nc.sync.dma_start(st_in[:], stv[:])
            nc.gpsimd.collective_compute(
                "AllReduce", Alu.add, replica_groups=RG,
                ins=[st_in.opt()], outs=[st_out.opt()])

            # residual matmuls overlap the stats AllReduce wait; the
            # accumulation group stays open for the rank-1 mean-shift below
            rowsT = [tailp.tile([128, MY_N], bf16, tag=f"rowsT{fc}", name=f"rowsT{fc}")
                     for fc in range(FC)]
            for fc in range(FC):
                nc.vector.tensor_add(rowsT[fc][:], rnaT[fc][:], disT[fc][:])
            r_ps = ps_vals.tile([128, MY_N], f32, tag="big", name="resid")
            for half in range(2):
                sl = slice(half * 512, (half + 1) * 512)
                for fc in range(FC):
                    nc.tensor.matmul(r_ps[:, sl], wrT[fc][:], rowsT[fc][:, sl],
                                     start=(fc == 0), stop=False)

            str_ = tailp.tile([1, 32], f32, tag="str", name="str")
            nc.sync.dma_start(str_[:], st_out[:])

            c = 1.0 / float(N * O)
            mu = tailp.tile([1, 1], f32, tag="mu", name="mu")
            nc.vector.tensor_scalar_mul(mu[:], str_[0:1, 0:1], c)
            m2 = tailp.tile([1, 1], f32, tag="m2", name="m2")
            nc.vector.tensor_scalar_mul(m2[:], str_[0:1, 16:17], c)
            mu2 = tailp.tile([1, 1], f32, tag="mu2", name="mu2")
            nc.vector.tensor_mul(mu2[:], mu[:], mu[:])
            var = tailp.tile([1, 1], f32, tag="var", name="var")
            nc.vector.tensor_sub(var[:], m2[:], mu2[:])
            vpe = tailp.tile([1, 1], f32, tag="vpe", name="vpe")
            nc.vector.tensor_scalar_add(vpe[:], var[:], EPS)
            sd = tailp.tile([1, 1], f32, tag="sd", name="sd")
            nc.scalar.activation(sd[:], vpe[:], Act.Sqrt)
            rstd = tailp.tile([1, 1], f32, tag="rstd", name="rstd")
            nc.vector.reciprocal(rstd[:], sd[:])
            negmurs = tailp.tile([1, 1], f32, tag="negmurs", name="negmurs")
            nc.vector.tensor_mul(negmurs[:], mu[:], rstd[:])
            nc.vector.tensor_scalar_mul(negmurs[:], negmurs[:], -1.0)

            a_col = tailp.tile([128, 1], f32, tag="acol", name="acol")
            nc.gpsimd.partition_broadcast(a_col[:], rstd[:])
            b_row = tailp.tile([1, 128], f32, tag="brow", name="brow")
            nc.scalar.activation(b_row[:], ones_row[0:1, 0:128], Act.Copy,
                                 scale=negmurs[:])

            for half in range(2):
                sl = slice(half * 512, (half + 1) * 512)
                nc.tensor.matmul(r_ps[:, sl], b_row[:], ones_row[:],
                                 start=False, stop=True)

            pre = tailp.tile([128, MY_N], f32, tag="pre", name="pre")
            nc.vector.scalar_tensor_tensor(pre[:], vs[:], a_col[:], r_ps[:],
                                           op0=Alu.mult, op1=Alu.add)
            negp = tailp.tile([128, MY_N], f32, tag="negp", name="negp")
            nc.vector.tensor_scalar_min(negp[:], pre[:], 0.0)
            w = tailp.tile([128, MY_N], f32, tag="w", name="w")
            nc.scalar.activation(w[:], negp[:], Act.Exp)
            r1 = tailp.tile([128, MY_N], f32, tag="r1", name="r1")
            nc.vector.tensor_scalar_max(r1[:], pre[:], 0.0)
            outt = tailp.tile([128, MY_N], f32, tag="outt", name="outt")
            nc.vector.scalar_tensor_tensor(outt[:], w[:], -1.0, r1[:],
                                           op0=Alu.add, op1=Alu.add)
            nc.sync.dma_start(out_dram, outt[:])
            tailp.release()

    nc.compile()
    return nc


def _get_nc():
    if "nc" not in _cached:
        _cached["nc"] = _build()
    return _cached["nc"]


def kernel(input_mat, connectivity_mask, proj_rna, proj_dis, score_src,
           score_tgt, residual_w):
    import ml_dtypes
    from concourse.bass_utils import run_bass_kernel_spmd

    bf16 = ml_dtypes.bfloat16
    nc = _get_nc()
    input_mat = np.asarray(input_mat, np.float32)
    connectivity_mask = np.asarray(connectivity_mask, np.float32)
    proj_rna = np.asarray(proj_rna, np.float32)
    proj_dis = np.asarray(proj_dis, np.float32)
    residual_w = np.asarray(residual_w, np.float32)

    ident = np.eye(128, dtype=np.float32)
    sel39 = (np.arange(128) < SPLIT_ROW).astype(np.float32)[:, None]
    rna_mask = (np.arange(N) < N_RNA).astype(np.float32)[:, None]
    in_rna_full = input_mat * rna_mask
    in_dis_full = input_mat * (1.0 - rna_mask)

    # layout-only host prep: transposes + bf16 casts
    inT = np.ascontiguousarray(input_mat.T).reshape(FC, 128, N).astype(bf16)
    projc_rna = np.ascontiguousarray(
        np.transpose(proj_rna, (1, 0, 2)).reshape(F, H * O)
    ).reshape(FC, 128, H * O).astype(bf16)
    projc_dis = np.ascontiguousarray(
        np.transpose(proj_dis, (1, 0, 2)).reshape(F, H * O)
    ).reshape(FC, 128, H * O).astype(bf16)
    wrT = np.ascontiguousarray(residual_w.T).reshape(FC, 128, O).astype(bf16)

    in_maps = []
    for k in range(N_CORES):
        r0, r1 = k * MY_N, (k + 1) * MY_N
        in_maps.append({
            "maskT": np.ascontiguousarray(connectivity_mask[r0:r1].T).astype(bf16),
            "inT": inT,
            "in_rnaT": np.ascontiguousarray(in_rna_full[r0:r1].T).reshape(
                FC, 128, MY_N).astype(bf16),
            "in_disT": np.ascontiguousarray(in_dis_full[r0:r1].T).reshape(
                FC, 128, MY_N).astype(bf16),
            "projc_rna": projc_rna,
            "projc_dis": projc_dis,
            "score_src": np.asarray(score_src, np.float32),
            "score_tgt": np.asarray(score_tgt, np.float32),
            "wrT": wrT,
            "identf": ident,
            "sel39": sel39,
            "invsel39": 1.0 - sel39,
        })

    res = run_bass_kernel_spmd(nc, in_maps, core_ids=list(range(N_CORES)))
    _cached["last_result"] = res
    out = np.empty((N, O), np.float32)
    for k in range(N_CORES):
        out[k * MY_N:(k + 1) * MY_N, :] = res.results[k]["out"].T
    return out
